# revision 22
# baseline (speedup 1.0000x reference)
"""Trainium2 (8 NeuronCores) kernel for a GPT-2 style causal attention block.

Reference math (per batch b):
    qkv = x @ W_attn + b_attn            # [T, 3E]
    q,k,v split -> heads H=16, D=64
    att = softmax(mask(q k^T / sqrt(D))) # causal mask
    y   = (att @ v) @ W_proj + b_proj    # [T, E]

Sharding (8 cores, no collectives):
    core c = (batch b = c//2, head-group hg = c%2 of 8 heads).
    Each core computes a PARTIAL y[b] = O_local @ W_proj[rows of its heads].
    Host sums the two partials per batch and adds b_proj (exact, commutes).

Device kernel per core (all bf16 matmuls, fp32 PSUM accumulate):
    phase 1: Q^T, K^T (feats on partitions) and V (rows on partitions) via
             matmuls from host-fed x^T and W shards.  1/sqrt(D) is folded
             into the Q columns of W on the host (exact: /8 is a pow2).
    phase 2: per (head-pair, q-chunk of 512): S^T tiles [128 k, 512 q] on
             PE, exp on ACT (no max-subtraction needed: scores are O(1) by
             construction).  The two heads' K=64 S^T matmuls are packed into
             one PSUM tile [kpos, j, q] and emitted back-to-back: they land
             on different PE row halves (tile_position auto-derived from the
             kT/qT base partition) and different PSUM banks, so each k-tile's
             S pair runs CONCURRENTLY on the array (~2x on the S stream).
             Causal structure: k-tiles above the diagonal are skipped,
             matmuls/exp on diagonal tiles are trimmed to live columns, and
             the 128-wide mixed band is masked by multiplying with a 128x128
             triangular tile.  O'^T accumulates with a V' that has a
             ones-column appended -> row 64 of O' is the softmax
             denominator.  Normalization happens off-PSUM: one [65,512] copy,
             DRAM-bounce broadcast of the denominator row, fast reciprocal,
             multiply into O^T.
    phase 3: y_partial = O @ W_proj_shard, PSUM -> SBUF -> DRAM (f32).

Schedule (the real perf lever on HW): the Tile scheduler is a per-engine
priority heap (priority = emission order) gated by readiness.  The
attention phases are ACT(exp)-paced, and any PE micro-idle risks the HAM
clock gate re-throttling the PE to 1.2 GHz (the original schedule lost
~45us to one 75us cold stretch).  So the dense matmul work (V, QK, proj)
is staggered across the attention chunks as filler: minimal prologue
(V rt0-3 + qk0 rc0), V tail + qk0 tail + qk1 into attn(0), qk2 into
attn(1), qk3 (reversed rc to match attn(3)'s descending qc order) into
attn(2), proj per-chunk into attn(3).  Early-phase PSUM->SBUF copies run
on the (exp-idle) scalar engine so the DVE has no copy backlog that would
throttle the psA slot turnaround when attention starts.
"""

import os
import numpy as np
import ml_dtypes

B, T, E, H = 4, 2048, 1024, 16
D = E // H            # 64
NCORES = 8
HL = H // 2           # local heads per core
DL = HL * D           # 512 local attention feats
QC = 512              # q-chunk width
NQC = T // QC         # 4
NKT = T // 128        # 16 k-tiles
P = 128

BF16 = ml_dtypes.bfloat16

_graph_cache = {}
LAST_RESULT = None    # BassKernelResults of the most recent run (for test.py)


def _build(causal: bool, with_bias: bool):
    import concourse.bass as bass  # noqa: F401
    import concourse.tile as tile
    from concourse import bacc, mybir
    from concourse.masks import make_upper_triangular

    bf16 = mybir.dt.bfloat16
    f32 = mybir.dt.float32
    Exp = mybir.ActivationFunctionType.Exp

    KIN = 1152 if with_bias else 1024   # qkv contraction (pad bias row to a full tile)
    NKIN = KIN // P

    nc = bacc.Bacc("TRN2", target_bir_lowering=False, debug=False,
                   num_devices=NCORES)
    xT = nc.declare_dram_parameter("xT", [KIN, T], bf16, isOutput=False)
    wqkv = nc.declare_dram_parameter("wqkv", [KIN, 3 * DL], bf16, isOutput=False)
    wproj = nc.declare_dram_parameter("wproj", [DL, E], bf16, isOutput=False)
    if not causal:
        maskT = nc.declare_dram_parameter("maskT", [T, T], bf16, isOutput=False)
    out = nc.declare_dram_parameter("out", [T, E], f32, isOutput=True)

    with tile.TileContext(nc) as tc, \
         tc.tile_pool(name="persist", bufs=1) as persist:
        # ---- persistent SBUF tensors ----
        xT_sb = persist.tile([P, NKIN, T], bf16, tag="xT_sb", name="xT_sb")
        wq_sb = persist.tile([P, NKIN, 3 * DL], bf16, tag="wq_sb", name="wq_sb")
        wp_sb = persist.tile([P, 4, E], bf16, tag="wp_sb", name="wp_sb")
        qT_sb = persist.tile([P, 4, T], bf16, tag="qT_sb", name="qT_sb")
        kT_sb = persist.tile([P, 4, T], bf16, tag="kT_sb", name="kT_sb")
        vP_sb = persist.tile([P, NKT, HL, D + 1], bf16, tag="vP_sb", name="vP_sb")
        oT_sb = persist.tile([P, 4, T], bf16, tag="oT_sb", name="oT_sb")
        band = persist.tile([P, P], bf16, tag="band", name="band")

        # spread input loads across engine DGE queues for a faster ramp
        # (scalar stays free: it runs the prologue-phase PSUM->SBUF copies
        # while ACT is otherwise idle, then the exp stream)
        dma_engines = [nc.sync, nc.gpsimd]
        di = 0

        def dma_in(out_ap, in_ap):
            nonlocal di
            dma_engines[di % len(dma_engines)].dma_start(out=out_ap, in_=in_ap)
            di += 1

        # DMA order is tuned so the first V matmuls (rt 0-3, needing wv[kt]
        # and xT[:, kt, 0:512] for every kt) can start ~6us in, and
        # qk(0, rc=0) (wq/wk g=0 cols + same xT) right after.
        for kt in range(NKIN):
            dma_in(wq_sb[:, kt, 2 * DL:3 * DL],
                   wqkv[kt * P:(kt + 1) * P, 2 * DL:3 * DL])
            dma_in(xT_sb[:, kt, 0:QC], xT[kt * P:(kt + 1) * P, 0:QC])
        for kt in range(NKIN):
            dma_in(wq_sb[:, kt, 0:P], wqkv[kt * P:(kt + 1) * P, 0:P])
            dma_in(wq_sb[:, kt, DL:DL + P],
                   wqkv[kt * P:(kt + 1) * P, DL:DL + P])
        for kt in range(NKIN):
            dma_in(xT_sb[:, kt, QC:2 * QC], xT[kt * P:(kt + 1) * P, QC:2 * QC])
        for g in range(1, 4):
            for kt in range(NKIN):
                dma_in(wq_sb[:, kt, g * P:(g + 1) * P],
                       wqkv[kt * P:(kt + 1) * P, g * P:(g + 1) * P])
                dma_in(wq_sb[:, kt, DL + g * P:DL + (g + 1) * P],
                       wqkv[kt * P:(kt + 1) * P, DL + g * P:DL + (g + 1) * P])
        for kt in range(NKIN):
            dma_in(xT_sb[:, kt, 2 * QC:], xT[kt * P:(kt + 1) * P, 2 * QC:])
        for g in range(4):
            dma_in(wp_sb[:, g, :], wproj[g * P:(g + 1) * P, :])
        if causal:
            # band[kp, qf] = 1.0 where kp <= qf else 0  (keep k <= q)
            make_upper_triangular(nc, band[:, :], val=1.0, diag=True)
        nc.vector.memset(vP_sb[:, :, :, D:D + 1], 1.0)
        # preload the ACT exp spline table so the first real exp does not
        # pay the table-switch latency mid-attention
        nc.scalar.activation(out=oT_sb[0:1, 0, 0:1],
                             in_=vP_sb[0:1, 0, 0, D:D + 1], func=Exp)

        with (
            tc.tile_pool(name="psA", bufs=2, space="PSUM") as psA,
            tc.tile_pool(name="psS", bufs=2, space="PSUM") as psS,
            tc.tile_pool(name="psO", bufs=2, space="PSUM") as psO,
            tc.tile_pool(name="sbw", bufs=6) as sbw,
            tc.tile_pool(name="sbm", bufs=4) as sbm,
            tc.tile_pool(name="drp", bufs=2, space="DRAM") as drp,
        ):
            def emit_v(rts, cpy=None):
                # ---- phase 1a: V = x @ Wv  (rows on partitions) ----
                cpy = cpy or nc.vector.tensor_copy
                for rt in rts:
                    ps_v = psA.tile([P, DL], f32, tag="mm512", name="ps_v")
                    for kt in range(NKIN):
                        nc.tensor.matmul(
                            ps_v[:],
                            lhsT=xT_sb[:, kt, rt * P:(rt + 1) * P],
                            rhs=wq_sb[:, kt, 2 * DL:3 * DL],
                            start=(kt == 0), stop=(kt == NKIN - 1))
                    cpy(vP_sb[:, rt, :, 0:D],
                        ps_v[:].rearrange("p (h d) -> p h d", h=HL))

            def emit_qk(g, rcs=None, cpy=None):
                # ---- phase 1b: Q^T, K^T for head-pair g ----
                cpy = cpy or nc.vector.tensor_copy
                for rc in (range(NQC) if rcs is None else rcs):
                    ps_q = psA.tile([P, QC], f32, tag="mm512", name="ps_q")
                    for kt in range(NKIN):
                        nc.tensor.matmul(
                            ps_q[:],
                            lhsT=wq_sb[:, kt, g * P:(g + 1) * P],
                            rhs=xT_sb[:, kt, rc * QC:(rc + 1) * QC],
                            start=(kt == 0), stop=(kt == NKIN - 1))
                    cpy(qT_sb[:, g, rc * QC:(rc + 1) * QC], ps_q[:])
                    ps_k = psA.tile([P, QC], f32, tag="mm512", name="ps_k")
                    for kt in range(NKIN):
                        nc.tensor.matmul(
                            ps_k[:],
                            lhsT=wq_sb[:, kt, DL + g * P:DL + (g + 1) * P],
                            rhs=xT_sb[:, kt, rc * QC:(rc + 1) * QC],
                            start=(kt == 0), stop=(kt == NKIN - 1))
                    cpy(kT_sb[:, g, rc * QC:(rc + 1) * QC], ps_k[:])

            def emit_proj(rts):
                # ---- phase 3: y_partial = O @ W_proj_shard for row tiles ----
                for rt in rts:
                    for nb in range(2):
                        ps_y = psA.tile([P, 512], f32, tag="mm512", name="ps_y")
                        for g in range(4):
                            nc.tensor.matmul(
                                ps_y[:],
                                lhsT=oT_sb[:, g, rt * P:(rt + 1) * P],
                                rhs=wp_sb[:, g, nb * 512:(nb + 1) * 512],
                                start=(g == 0), stop=(g == 3))
                        y_sb = sbw.tile([P, 512], f32, tag="y_sb", name="y_sb")
                        nc.vector.tensor_copy(y_sb[:], ps_y[:])
                        (nc.sync if (rt + nb) % 2 else nc.gpsimd).dma_start(
                            out=out[rt * P:(rt + 1) * P, nb * 512:(nb + 1) * 512],
                            in_=y_sb[:])

            def emit_attn_qc(g, qc):
                # ---- phase 2: attention for heads 2g, 2g+1, one q-chunk ----
                # O' matmuls are software-pipelined one k-group behind the
                # S^T matmuls: while ACT runs exp(k), the PE has O'(k-1)
                # ready to run.
                # Each PSUM S^T tile packs BOTH heads' slab for one k-tile as
                # [kpos, j, q]: the two heads' K=64 matmuls land on different
                # PE row halves (tile_position auto-derived from the kT/qT
                # base partition) and different PSUM banks (col 512 is the
                # bank boundary), share one exp-release gate, and are emitted
                # back-to-back -- so each k-tile's S pair runs CONCURRENTLY
                # on the PE array instead of serializing.
                if True:
                    nkt = 4 * (qc + 1) if causal else NKT
                    ps_o = [psO.tile([P, QC], f32, tag="ps_o", name=f"ps_o{j}") for j in range(2)]

                    def emit_o(kt2, pT, ss):
                        # j-inner order alternates the two psO banks so each
                        # matmul's drain overlaps the next one's stream
                        for t2 in range(2):
                            for j in range(2):
                                kt = 2 * kt2 + t2
                                nc.tensor.matmul(
                                    ps_o[j][0:D + 1, ss[t2]:],
                                    lhsT=vP_sb[:, kt, 2 * g + j, :],
                                    rhs=pT[:, t2, j, ss[t2]:],
                                    start=(kt == 0), stop=(kt == nkt - 1))

                    prev = None
                    for kt2 in range(nkt // 2):
                        # live-column start per slab (diagonal tiles are
                        # fully masked below column kt*128 - qc*512)
                        ss = [max(0, (2 * kt2 + t2) * P - qc * QC) if causal else 0
                              for t2 in range(2)]
                        # per-k-tile PSUM tile packs both heads: [kpos, j, q]
                        ps_ss = [psS.tile([P, 2, QC], f32, tag="ps_s",
                                          name=f"ps_s{t2}") for t2 in range(2)]
                        for t2 in range(2):
                            kt = 2 * kt2 + t2
                            for j in range(2):
                                nc.tensor.matmul(
                                    ps_ss[t2][:, j, ss[t2]:],
                                    lhsT=kT_sb[j * D:(j + 1) * D, g, kt * P:(kt + 1) * P],
                                    rhs=qT_sb[j * D:(j + 1) * D, g,
                                              qc * QC + ss[t2]:(qc + 1) * QC],
                                    start=True, stop=True)
                        # pT layout [kpos, t2, j, q]
                        pT = sbw.tile([P, 2, 2, QC], bf16, tag="pT", name="pT")
                        if prev is not None:
                            emit_o(*prev)
                            prev = None
                        for t2 in range(2):
                            kt = 2 * kt2 + t2
                            s = ss[t2]
                            nc.scalar.activation(out=pT[:, t2, :, s:],
                                                 in_=ps_ss[t2][:, :, s:],
                                                 func=Exp)
                            if causal:
                                if kt >= 4 * qc:  # diagonal-band k-tile
                                    for j in range(2):
                                        nc.vector.tensor_mul(
                                            pT[:, t2, j, s:s + P],
                                            pT[:, t2, j, s:s + P],
                                            band[:, :])
                            else:
                                msk = sbm.tile([P, QC], bf16, tag="msk", name="msk")
                                nc.sync.dma_start(
                                    out=msk[:],
                                    in_=maskT[kt * P:(kt + 1) * P, qc * QC:(qc + 1) * QC])
                                for j in range(2):
                                    nc.vector.tensor_mul(pT[:, t2, j, :],
                                                         pT[:, t2, j, :], msk[:])
                        prev = (kt2, pT, ss)
                    emit_o(*prev)
                    for j in range(2):
                        # early-release ps_o: copy O + rowsum to SBUF in one
                        # shot, then normalize off-PSUM:  O[d, q] / rowsum[q]
                        oU = sbm.tile([D + 1, QC], f32, tag="oU", name="oU")
                        nc.vector.tensor_copy(oU[:], ps_o[j][0:D + 1, :])
                        rdr = drp.tile([1, QC], f32, tag="rdr", name="rdr")
                        nc.sync.dma_start(out=rdr[:], in_=oU[D:D + 1, :])
                        rb = sbm.tile([D, QC], f32, tag="rb", name="rb")
                        nc.sync.dma_start(out=rb[:], in_=rdr[:].to_broadcast((D, QC)))
                        nc.vector.reciprocal_approx_fast(out=rb[:], in_=rb[:])
                        nc.vector.tensor_mul(
                            oT_sb[j * D:(j + 1) * D, g, qc * QC:(qc + 1) * QC],
                            oU[0:D, :], rb[:])
            # emission schedule: the Tile scheduler is a per-engine priority
            # heap (priority = emission order) gated by readiness, so dense
            # matmuls emitted anywhere after a point act as PE filler for the
            # exp-bound attention stream.  The attention phases are ACT-bound
            # (exp deficit ~12us per head-pair); if the PE micro-idles with
            # no ready dense work the HAM clock-gate re-throttles it to
            # 1.2 GHz (baseline lost ~45us to one 75us cold stretch).  So:
            # keep the dense prologue minimal and stagger every remaining
            # dense group across the attention chunks so filler never runs
            # dry: V tail + qk0 tail + qk1 into attn(0), qk2 into attn(1),
            # qk3 (reversed rc, matching attn(3)'s descending qc order) into
            # attn(2), proj per-chunk into attn(3).
            # V and qk0 copies run on the (otherwise idle) scalar engine so
            # the early dense burst leaves no DVE copy backlog to throttle
            # the psA slot turnaround once attention starts
            emit_v(range(0, 4), cpy=nc.scalar.copy)
            emit_qk(0, rcs=[0], cpy=nc.scalar.copy)
            for qc in range(NQC):
                emit_attn_qc(0, qc)
                if qc < NQC - 1:
                    emit_v(range(4 * qc + 4, 4 * qc + 8), cpy=nc.scalar.copy)
                    emit_qk(0, rcs=[qc + 1], cpy=nc.scalar.copy)
                emit_qk(1, rcs=[qc])
            for g in (1, 2):
                for qc in range(NQC):
                    emit_attn_qc(g, qc)
                    emit_qk(g + 1, rcs=[qc if g == 1 else NQC - 1 - qc])
            for qc in range(NQC - 1, -1, -1):
                emit_attn_qc(3, qc)
                emit_proj(range(4 * qc, 4 * qc + 4))

    nc.compile()
    return nc


def _get_graph(causal: bool, with_bias: bool):
    key = (causal, with_bias)
    if key not in _graph_cache:
        _graph_cache[key] = _build(causal, with_bias)
    return _graph_cache[key]


def make_in_maps(x, mask, W_attn, b_attn, W_proj, b_proj, causal, with_bias):
    """Host-side sharding: per-core input dict (bf16)."""
    in_maps = []
    maskT_bf = None
    if not causal:
        m = np.asarray(mask).reshape(T, T)
        maskT_bf = np.ascontiguousarray(m.T).astype(BF16)
    for c in range(NCORES):
        b, hg = c // 2, c % 2
        lo, hi = hg * DL, (hg + 1) * DL
        Wq = W_attn[:, lo:hi] * np.float32(0.125)
        Wk = W_attn[:, E + lo:E + hi]
        Wv = W_attn[:, 2 * E + lo:2 * E + hi]
        wqkv = np.concatenate([Wq, Wk, Wv], axis=1).astype(np.float32)
        xt = np.ascontiguousarray(x[b].T).astype(np.float32)
        if with_bias:
            brow = np.concatenate([
                b_attn[lo:hi] * np.float32(0.125),
                b_attn[E + lo:E + hi],
                b_attn[2 * E + lo:2 * E + hi]]).astype(np.float32)
            wqkv = np.concatenate(
                [wqkv, brow[None, :], np.zeros((P - 1, 3 * DL), np.float32)], axis=0)
            xt = np.concatenate(
                [xt, np.ones((1, T), np.float32), np.zeros((P - 1, T), np.float32)],
                axis=0)
        im = {
            "xT": np.ascontiguousarray(xt).astype(BF16),
            "wqkv": np.ascontiguousarray(wqkv).astype(BF16),
            "wproj": np.ascontiguousarray(W_proj[lo:hi, :]).astype(BF16),
        }
        if not causal:
            im["maskT"] = maskT_bf
        in_maps.append(im)
    return in_maps


def expected_partial(x, mask, W_attn, b_attn, W_proj, core):
    """Numpy reference for ONE core's partial output (for sim testing)."""
    b, hg = core // 2, core % 2
    lo, hi = hg * DL, (hg + 1) * DL
    q = x[b] @ W_attn[:, lo:hi] + b_attn[lo:hi]
    k = x[b] @ W_attn[:, E + lo:E + hi] + b_attn[E + lo:E + hi]
    v = x[b] @ W_attn[:, 2 * E + lo:2 * E + hi] + b_attn[2 * E + lo:2 * E + hi]
    q = q.reshape(T, HL, D)
    k = k.reshape(T, HL, D)
    v = v.reshape(T, HL, D)
    att = np.einsum('qhd,khd->hqk', q, k) / np.sqrt(D)
    m = np.asarray(mask).reshape(T, T)
    att = np.where(m[None] == 0, np.float32(-1e20), att)
    att = att - att.max(axis=-1, keepdims=True)
    att = np.exp(att)
    att = att / att.sum(axis=-1, keepdims=True)
    o = np.einsum('hqk,khd->qhd', att, v).reshape(T, DL)
    return o @ W_proj[lo:hi, :]


def kernel(x, mask, W_attn, b_attn, W_proj, b_proj):
    global LAST_RESULT
    from concourse.bass_utils import run_bass_kernel_spmd

    x = np.asarray(x, dtype=np.float32)
    W_attn = np.asarray(W_attn, dtype=np.float32)
    b_attn = np.asarray(b_attn, dtype=np.float32)
    W_proj = np.asarray(W_proj, dtype=np.float32)
    b_proj = np.asarray(b_proj, dtype=np.float32)

    mask2d = np.asarray(mask).reshape(T, T)
    causal = bool(np.array_equal(mask2d != 0, np.tril(np.ones((T, T), bool))))
    if not causal and not (mask2d != 0).any(axis=1).all():
        # A fully-masked query row: reference softmax degenerates to uniform
        # attention; not representable in the 0/1-multiply fast path.  This
        # cannot occur for the causal mask; fall back to exact host math.
        y = np.stack([
            sum(expected_partial(x, mask, W_attn, b_attn, W_proj, 2 * b + hg)
                for hg in range(2))
            for b in range(B)]).astype(np.float32)
        return y + b_proj
    with_bias = bool(np.any(b_attn))

    nc = _get_graph(causal, with_bias)
    in_maps = make_in_maps(x, mask, W_attn, b_attn, W_proj, b_proj,
                           causal, with_bias)
    trace = bool(int(os.environ.get("CK_TRACE", "0")))
    res = run_bass_kernel_spmd(nc, in_maps, core_ids=list(range(NCORES)),
                               trace=trace)
    LAST_RESULT = res
    y = np.empty((B, T, E), np.float32)
    for b in range(B):
        y[b] = res.results[2 * b]["out"].astype(np.float32) \
             + res.results[2 * b + 1]["out"].astype(np.float32)
    return y + b_proj

